# revision 4
# baseline (speedup 1.0000x reference)
"""Per-expert SwiGLU FFN (MoE) for Trainium2, expert-parallel over 8 cores.

Reference (per expert e):
    y1 = x[e] @ W_fc1[e]; y2 = x[e] @ W_fc2[e]
    out[e] = (silu(y1) * y2) @ W_fc3[e]
Shapes: E=8, T=1024, D=2048, H=5632. One expert per core.

The dominant per-call cost on this stack is input staging, which scales
with declared input bytes (~12.5 GB/s). So the host casts everything to
fp16 (halving bytes) and pre-tiles the weights so each device DMA is one
big fully-contiguous transfer (128 descriptors, 32-88KB per partition
run), with no on-chip transposes or dtype casts:

  xt  [128, 16, 1024]        xt[p, dt, t] = x[t, 128*dt+p]        (x^T tiled)
  w12 [128, 11, 16, 2, 512]  w12[p,c,dt,w,h] = W{w}[128*dt+p, 512*c+h]
  w3t [128, 2, 44, 1024]     w3t[p,hf,ht,d] = W3[128*ht+p, 1024*hf+d]

Phase A (per h-chunk c of 512 cols): 1 DMA brings W1+W2 columns for all
of D; per h-tile, 16 fp16 matmuls accumulate over d into PSUM for each
(y1, y2, T-half); silu on ScalarE, mul on VectorE -> resident y strip
[128, T] fp16.  Phase B (per d-half): 1 DMA brings W3 rows for all of H;
per t-tile, 44 fp16 matmuls accumulate over h into PSUM; evict fp16.
Output is fp16 on device, upcast to fp32 on host.
"""

import numpy as np

import concourse.mybir as mybir
import concourse.tile as tile
from concourse import bacc
from concourse.bass_utils import run_bass_kernel_spmd

E, T, D, H = 8, 1024, 2048, 5632
P = 128
DT = D // P    # 16 d-tiles
HT = H // P    # 44 h-tiles
TT = T // P    # 8 t-tiles
HC = 512       # phase-A h-chunk width
NHC = H // HC  # 11 chunks
HPC = HC // P  # 4 h-tiles per chunk
DH = 1024      # phase-B d-half width

F32 = mybir.dt.float32
F16 = mybir.dt.float16

_cache = {}


def _build():
    nc = bacc.Bacc("TRN2", target_bir_lowering=False, debug=False)
    xt = nc.dram_tensor("xt", [P, DT * T], F16, kind="ExternalInput").ap()
    w12 = nc.dram_tensor("w12", [P, NHC * DT * 2 * HC], F16, kind="ExternalInput").ap()
    w3t = nc.dram_tensor("w3t", [P, 2 * HT * DH], F16, kind="ExternalInput").ap()
    out = nc.dram_tensor("out", [T, D], F16, kind="ExternalOutput").ap()

    with tile.TileContext(nc) as tc:
        with tc.tile_pool(name="y", bufs=1) as ypool:
            y_sb = [ypool.tile([P, T], F16, name=f"y{h}", tag=f"y{h}") for h in range(HT)]

            # ---------------- Phase A ----------------
            with (
                tc.tile_pool(name="xT", bufs=1) as xpool,
                tc.tile_pool(name="wa", bufs=2) as wpool,
                tc.tile_pool(name="s1", bufs=2) as spool,
                tc.tile_pool(name="psA", bufs=2, space="PSUM") as psA,
            ):
                xts = xpool.tile([P, DT, T], F16)
                nc.sync.dma_start(xts[:], xt.rearrange("p (dt t) -> p dt t", dt=DT))

                for c in range(NHC):
                    wa = wpool.tile([P, DT, 2, HC], F16, name="wa", tag="wa")
                    nc.sync.dma_start(
                        wa[:],
                        w12[:, c * DT * 2 * HC:(c + 1) * DT * 2 * HC].rearrange(
                            "p (dt w h) -> p dt w h", dt=DT, w=2))
                    for i in range(HPC):
                        h = c * HPC + i
                        hs = slice(i * P, (i + 1) * P)
                        y1 = psA.tile([P, T], F32, name="y1", tag="ps1")
                        y2 = psA.tile([P, T], F32, name="y2", tag="ps2")
                        for half in range(2):
                            th = slice(half * 512, (half + 1) * 512)
                            for d in range(DT):
                                nc.tensor.matmul(
                                    y1[:, th], lhsT=wa[:, d, 0, hs], rhs=xts[:, d, th],
                                    start=(d == 0), stop=(d == DT - 1))
                            for d in range(DT):
                                nc.tensor.matmul(
                                    y2[:, th], lhsT=wa[:, d, 1, hs], rhs=xts[:, d, th],
                                    start=(d == 0), stop=(d == DT - 1))
                        s1 = spool.tile([P, T], F16, name="s1", tag="s1")
                        nc.scalar.activation(
                            s1[:], y1[:], mybir.ActivationFunctionType.Silu)
                        nc.vector.tensor_mul(y_sb[h][:], s1[:], y2[:])

            # ---------------- Phase B ----------------
            with (
                tc.tile_pool(name="w3", bufs=1) as w3pool,
                tc.tile_pool(name="outs", bufs=4) as opool,
                tc.tile_pool(name="psB", bufs=4, space="PSUM") as psB,
            ):
                for hf in range(2):
                    w3h = w3pool.tile([P, HT, DH], F16, name="w3h", tag="w3h")
                    nc.sync.dma_start(
                        w3h[:],
                        w3t[:, hf * HT * DH:(hf + 1) * HT * DH].rearrange(
                            "p (ht d) -> p ht d", ht=HT))
                    for ts in range(TT):
                        tsl = slice(ts * P, (ts + 1) * P)
                        ob = opool.tile([P, DH], F16, name="ob", tag="ob")
                        for q in range(2):
                            qs = slice(q * 512, (q + 1) * 512)
                            po = psB.tile([P, 512], F32, name="po", tag="po")
                            for h in range(HT):
                                nc.tensor.matmul(
                                    po[:], lhsT=y_sb[h][:, tsl], rhs=w3h[:, h, qs],
                                    start=(h == 0), stop=(h == HT - 1))
                            nc.scalar.activation(
                                ob[:, qs], po[:], mybir.ActivationFunctionType.Copy)
                        nc.sync.dma_start(out[tsl, hf * DH:(hf + 1) * DH], ob[:])

    nc.compile()
    return nc


def prep_inputs(x, W_fc1, W_fc2, W_fc3):
    """Host-side cast to fp16 + pre-tiling into the DMA-friendly layouts."""
    in_maps = []
    for e in range(E):
        xe = np.asarray(x[e], dtype=np.float16)
        w1 = np.asarray(W_fc1[e], dtype=np.float16)
        w2 = np.asarray(W_fc2[e], dtype=np.float16)
        w3 = np.asarray(W_fc3[e], dtype=np.float16)

        # xt[p, dt, t] = x[t, dt*128+p]
        xt = np.ascontiguousarray(
            xe.T.reshape(DT, P, T).transpose(1, 0, 2)).reshape(P, DT * T)

        # w12[p, c, dt, w, h'] = W{w}[dt*128+p, c*512+h']
        w1r = w1.reshape(DT, P, NHC, HC)
        w2r = w2.reshape(DT, P, NHC, HC)
        w12 = np.ascontiguousarray(
            np.stack([w1r, w2r], axis=3).transpose(1, 2, 0, 3, 4)
        ).reshape(P, NHC * DT * 2 * HC)

        # w3t[p, hf, ht, d'] = W3[ht*128+p, hf*1024+d']
        w3r = w3.reshape(HT, P, 2, DH)
        w3tt = np.ascontiguousarray(w3r.transpose(1, 2, 0, 3)).reshape(P, 2 * HT * DH)

        in_maps.append({"xt": xt, "w12": w12, "w3t": w3tt})
    return in_maps


def _make_runner(nc):
    """PJRT exec path without output-buffer donation: the donated zero
    output buffers in run_bass_kernel_spmd's axon path are staged as extra
    inputs every call; skipping them shaves per-call input bytes."""
    import jax
    from jax.sharding import Mesh, PartitionSpec
    try:
        from jax.experimental.shard_map import shard_map
    except ImportError:
        from jax import shard_map
    from concourse import bass2jax as b2j

    b2j.install_neuronx_cc_hook()
    partition_name = nc.partition_id_tensor.name if nc.partition_id_tensor else None

    in_names, out_names, out_avals = [], [], []
    for alloc in nc.m.functions[0].allocations:
        if not isinstance(alloc, mybir.MemoryLocationSet):
            continue
        name = alloc.memorylocations[0].name
        if alloc.kind == "ExternalInput":
            if name != partition_name:
                in_names.append(name)
        elif alloc.kind == "ExternalOutput":
            out_names.append(name)
            out_avals.append(jax.core.ShapedArray(
                tuple(alloc.tensor_shape), mybir.dt.np(alloc.dtype)))

    all_in_names = list(in_names) + list(out_names)
    if partition_name is not None:
        all_in_names.append(partition_name)

    def _body(*args):
        operands = list(args)
        if partition_name is not None:
            operands.append(b2j.partition_id_tensor())
        return tuple(b2j._bass_exec_p.bind(
            *operands,
            out_avals=tuple(out_avals),
            in_names=tuple(all_in_names),
            out_names=tuple(out_names),
            lowering_input_output_aliases=(),
            sim_require_finite=True,
            sim_require_nnan=True,
            nc=nc,
        ))

    devices = jax.devices()[:E]
    mesh = Mesh(np.asarray(devices), ("core",))
    fn = jax.jit(
        shard_map(_body, mesh=mesh,
                  in_specs=(PartitionSpec("core"),) * len(in_names),
                  out_specs=(PartitionSpec("core"),) * len(out_names),
                  check_rep=False),
        keep_unused=True,
    )
    return fn, in_names, out_names, out_avals


def _run(nc, in_maps):
    """Execute on 8 cores; returns list of per-core {name: np.ndarray}."""
    import jax

    if "runner" not in _cache:
        _cache["runner"] = _make_runner(nc)
    fn, in_names, out_names, out_avals = _cache["runner"]
    args = [
        np.concatenate([in_maps[e][n] for e in range(E)], axis=0)
        for n in in_names
    ]
    outs = fn(*args)
    return [
        {
            name: np.asarray(outs[i]).reshape(E, *out_avals[i].shape)[e]
            for i, name in enumerate(out_names)
        }
        for e in range(E)
    ]


def kernel(x, W_fc1, W_fc2, W_fc3, trace=False, trace_cores=None):
    if "nc" not in _cache:
        _cache["nc"] = _build()
    nc = _cache["nc"]

    in_maps = prep_inputs(x, W_fc1, W_fc2, W_fc3)
    if trace:
        res = run_bass_kernel_spmd(
            nc, in_maps, core_ids=list(range(E)),
            trace=trace, trace_cores=trace_cores,
        )
        kernel.last_result = res
        results = res.results
    else:
        try:
            results = _run(nc, in_maps)
        except Exception:
            res = run_bass_kernel_spmd(nc, in_maps, core_ids=list(range(E)))
            results = res.results
    return np.stack([results[e]["out"] for e in range(E)]).astype(np.float32)
